# revision 43
# baseline (speedup 1.0000x reference)
"""Trainium2 8-core kernel for nn_AttentionMechanism_51049981281163.

Reference module: multi-head attention, B=2, S=2048, D=1024, H=16 heads,
head_dim=64, fp32, mask all-ones, biases all-zero.

Sharding: batch x head-group tensor parallel. Core c handles batch b=c//4
and head group g=c%4 (4 heads = 256 of the 1024 hidden dims). Wq/Wk/Wv are
split column-wise, Wo row-wise; each core computes a partial [S, D] output
and the host sums the 4 partials per batch (the "unshard" of row-parallel
Wo) and adds bo.

Device kernel (per core), bf16 matmul inputs with fp32 PSUM accumulate:
  - host passes x[b].T pre-tiled, so projections need no on-device transpose
  - Phase A streams the V projection against the chunked xt DMA (PE does
    real work during the input transfer; no dummy warm-up matmuls), then
    runs Q/K through the same rotating 8-bank PSUM pool; PSUM->SBUF casts
    are split between ScalarE and VectorE
  - QT/KT in [head_dim, S] layout; V in [S, head_dim] layout with a ones
    column per head (softmax denominators ride along as context row 64)
  - scores computed transposed [k, q]; the two heads of a pair use T0/T8
    64-row PE tiles into one two-bank PSUM tile
  - softmax exp is SPLIT between engines: half the k-tiles use ScalarE
    exp, half use a VectorE Schraudolph bit-trick (affine into the bf16
    exponent field via fp32->int16 convert, bitcast to bf16); the mean
    relative error of the trick is calibrated to ~0 so softmax
    normalization cancels most of it
  - 1/denominator = exp(-ln(denom)) on ScalarE (Ln and Exp share one ACT
    table set -> no table reloads), then ONE K=2 block-diagonal matmul
    broadcasts both heads' reciprocal rows across partitions; the final
    normalization multiply reads that PSUM tile directly
  - output projection from the context (already in lhsT layout) x Wo shard
"""

import sys

sys.path.insert(0, "/opt/trn_rl_repo")

import numpy as np

B, S, D, H = 2, 2048, 1024, 16
HD = D // H          # 64
SCALE = HD ** -0.5
NCORES = 8
GROUPS = 4           # head groups (cores per batch)
HPG = H // GROUPS    # 4 heads per group/core
DL = HPG * HD        # 256 local hidden dims per core
VW = HD + 1          # V block width incl. ones column

# Schraudolph exp in bf16 bit domain: i16 = x*SCALE*128*log2(e) + bias;
# bitcast(i16) ~= exp(x*SCALE).  Bias centered between round/trunc convert
# semantics; mean rel err ~0 (cancels in softmax), rms ~1.8%.
_LOG2E = 1.4426950408889634
EXP_A = SCALE * 128.0 * _LOG2E          # 23.0831...
EXP_B = 127.0 * 128.0 - 7.15            # 16248.85
# every score tile's exp is split by columns: ScalarE does [0, ACT_COLS)
# exactly, VectorE does the rest via the bit-trick — both engines work on
# the SAME k-tile concurrently at ~0.6us/tile, so exp always completes
# before the in-order PE queue reaches the context matmuls
ACT_COLS = 576
# Newton reciprocal: seed 1/d ~= bitcast(K - bits(d)), then r*(2 - d*r)
RCP_K = float(0x7EF0A3D7)
# context matmuls trail the scores stream by PIPE k-tiles so the exp
# latency (including semaphore propagation) never stalls the in-order
# PE queue
PIPE = 3


def _build_graph():
    import concourse.tile as tile
    from concourse import bacc, mybir

    F32 = mybir.dt.float32
    BF16 = mybir.dt.bfloat16
    I16 = mybir.dt.int16
    Exp = mybir.ActivationFunctionType.Exp
    Ln = mybir.ActivationFunctionType.Ln
    mult = mybir.AluOpType.mult
    add = mybir.AluOpType.add

    nc = bacc.Bacc("TRN2")

    # x[b].T tiled: [p, c, s] = x[b][s, c*128+p]
    xt_e = nc.declare_dram_parameter("xt", [128, 8, S], BF16, isOutput=False)
    # W[:, gsl] tiled: [p, c, d] = W[c*128+p, g*256+d]
    wq_e = nc.declare_dram_parameter("wq", [128, 8, DL], BF16, isOutput=False)
    wk_e = nc.declare_dram_parameter("wk", [128, 8, DL], BF16, isOutput=False)
    wv_e = nc.declare_dram_parameter("wv", [128, 8, DL], BF16, isOutput=False)
    # Wo[gsl, :] tiled: [p, kc, dd] = Wo[g*256+kc*128+p, dd]
    wo_e = nc.declare_dram_parameter("wo", [128, 2, D], BF16, isOutput=False)
    out_e = nc.declare_dram_parameter("out", [S, D], BF16, isOutput=True)

    with nc.allow_low_precision(reason="bf16 compute, 2e-2 tolerance"), \
         tile.TileContext(nc) as tc:
        with tc.tile_pool(name="big", bufs=1) as big:
            xt_sb = big.tile([128, 8, S], BF16)
            wq_sb = big.tile([128, 8, DL], BF16)
            wk_sb = big.tile([128, 8, DL], BF16)
            wv_sb = big.tile([128, 8, DL], BF16)
            wo_sb = big.tile([128, 2, D], BF16)
            qt_sb = big.tile([128, 2, S], BF16)
            kt_sb = big.tile([128, 2, S], BF16)
            vp_sb = big.tile([128, 16, HPG, VW], BF16)
            ctx_sb = big.tile([128, 2, S], BF16)
            ones_sb = big.tile([1, 64], BF16)

            nc.vector.memset(ones_sb[:], 1.0)
            nc.vector.memset(vp_sb[:, :, :, HD], 1.0)

            # DMA order = consumption order: wv + x chunks feed the
            # streamed V projection, then Q/K/Wo weights. Inputs split
            # across the two HWDGE queues (sync + scalar) for bandwidth;
            # ScalarE is idle this early so the issue cost is free.
            nc.sync.dma_start(out=wv_sb[:], in_=wv_e[:])
            for c in range(8):
                eng = nc.sync if c % 2 == 0 else nc.scalar
                if c == 0:
                    # first chunk in 4 pieces so the first V matmuls can
                    # start as soon as ~a quarter of it lands
                    for p in range(4):
                        psl = slice(p * 512, (p + 1) * 512)
                        eng.dma_start(out=xt_sb[:, 0, psl],
                                      in_=xt_e[:, 0, psl])
                else:
                    eng.dma_start(out=xt_sb[:, c, :], in_=xt_e[:, c, :])
            nc.scalar.dma_start(out=wk_sb[:], in_=wk_e[:])
            nc.sync.dma_start(out=wq_sb[:], in_=wq_e[:])
            nc.scalar.dma_start(out=wo_sb[:], in_=wo_e[:])

            # ---- Phase A: projections through one rotating PSUM pool ----
            # V streams against the xt DMA: each arriving 128-row chunk of
            # x.T enables 16 matmuls, keeping PE busy during the transfer.
            with tc.tile_pool(name="pa", bufs=8, space="PSUM") as pa:
                pv = [pa.tile([128, 512], F32, tag="pa", name="pv")
                      for _ in range(8)]
                # one accumulation group per PSUM bank (start zeroes the
                # whole bank): the kt pair sharing a tile is one group
                for c in range(8):
                    for kt in range(16):
                        nc.tensor.matmul(
                            pv[kt // 2][:, (kt % 2) * 256:(kt % 2) * 256 + 256],
                            lhsT=xt_sb[:, c, kt * 128:(kt + 1) * 128],
                            rhs=wv_sb[:, c, :],
                            start=(c == 0 and kt % 2 == 0),
                            stop=(c == 7 and kt % 2 == 1),
                            skip_group_check=True,
                        )
                for kt in range(16):
                    src = pv[kt // 2][:, (kt % 2) * 256:(kt % 2) * 256 + 256]
                    dst = vp_sb[:, kt, :, 0:HD]
                    srcr = src.rearrange("p (h d) -> p h d", h=HPG)
                    if kt % 2 == 0:
                        nc.vector.tensor_copy(dst, srcr)
                    else:
                        nc.scalar.copy(dst, srcr)
                # K then Q through the same pool; allocations recycle banks
                # as the V (then K) casts drain. Two accumulation chains
                # are interleaved so consecutive matmuls hit different
                # PSUM banks (same-bank accumulation serializes at the
                # full fill+drain latency instead of the N-cycle rate).
                for w_sb, o_sb in ((wk_sb, kt_sb), (wq_sb, qt_sb)):
                    for m in range(2):
                        for h2 in range(2):
                            pp = [pa.tile([128, 512], F32, tag="pa",
                                          name="pp") for _ in range(2)]
                            for c in range(8):
                                for i in range(2):
                                    n4 = 2 * h2 + i
                                    nc.tensor.matmul(
                                        pp[i][:],
                                        lhsT=w_sb[:, c, m * 128:(m + 1) * 128],
                                        rhs=xt_sb[:, c,
                                                  n4 * 512:(n4 + 1) * 512],
                                        start=(c == 0), stop=(c == 7),
                                    )
                            for i in range(2):
                                n4 = 2 * h2 + i
                                dst = o_sb[:, m, n4 * 512:(n4 + 1) * 512]
                                if i == 0:
                                    nc.vector.tensor_copy(dst, pp[i][:])
                                else:
                                    nc.scalar.copy(dst, pp[i][:])

            # ---- Phase B: attention + output projection ----
            with tc.tile_pool(name="ps", bufs=3, space="PSUM") as pspool, \
                 tc.tile_pool(name="pc", bufs=2, space="PSUM") as pcpool, \
                 tc.tile_pool(name="pt", bufs=6) as ptpool, \
                 tc.tile_pool(name="cu", bufs=6) as cupool, \
                 tc.tile_pool(name="sm", bufs=4) as smpool, \
                 tc.tile_pool(name="ob", bufs=3) as obpool:
                def newton_recip(cus, eng=None):
                    # 1/denom for both heads: bit-hack seed r0 =
                    # bitcast(K - bits(d)) via fp32 affine on the int32 view
                    # + convert-on-write (VectorE), then one Newton step
                    # r0*(2 - d*r0) on the otherwise-idle GPSIMD engine ->
                    # ~0.2% worst case, squared from ~4%. Emitted inside the
                    # NEXT block's kt0-3 window, where the DVE has no exp
                    # tiles queued.
                    r0i = smpool.tile([1, 2, 512], mybir.dt.int32, tag="ri")
                    dc = smpool.tile([1, 2, 512], F32, tag="dc")
                    nu = smpool.tile([1, 2, 512], F32, tag="nu")
                    nv = smpool.tile([1, 2, 512], F32, tag="nv")
                    rinv = smpool.tile([1, 2, 512], BF16, tag="rinv")
                    for o in range(2):
                        # single-src DVE ops may cross partition bases;
                        # stage the denominator row at partition 0
                        nc.vector.tensor_copy(dc[0:1, o, :],
                                              cus[o][HD:HD + 1, :])
                    nc.vector.tensor_scalar(
                        r0i[:], dc[:].bitcast(mybir.dt.int32),
                        -1.0, RCP_K, mult, add)
                    r0 = r0i[:].bitcast(F32)
                    if eng is None:
                        eng = nc.gpsimd
                    if eng is nc.gpsimd:
                        eng.tensor_mul(nu[:], dc[:], r0)
                    else:
                        eng.tensor_tensor(out=nu[:], in0=dc[:], in1=r0,
                                          op=mult)
                    eng.tensor_scalar(nv[:], nu[:], -1.0, 2.0, mult, add)
                    if eng is nc.gpsimd:
                        eng.tensor_mul(rinv[:], r0, nv[:])
                    else:
                        eng.tensor_tensor(out=rinv[:], in0=r0, in1=nv[:],
                                          op=mult)
                    return rinv

                def block_loop(n2, qc, j, pend):
                    # heads 2j/2j+1, q-chunk of 512. Both heads' scores land
                    # in ONE [128,1024] psum tile (separate banks). The
                    # context matmuls for k-tile kt are emitted after the
                    # scores for kt+PIPE, so the exp latency (ScalarE or
                    # VectorE) is hidden behind later scores instead of
                    # stalling the in-order PE queue.
                    q0 = n2 * 1024 + qc * 512
                    qh = slice(q0, q0 + 512)
                    pcs = []
                    for _ in range(2):
                        pcs.append(pcpool.tile([HD + 1, 512], F32, tag="pc",
                                               name="pc"))
                    pts = {}

                    def scores_stage(kt):
                        ksl = slice(kt * 128, (kt + 1) * 128)
                        ps = pspool.tile([128, 1024], F32, tag="ps", name="ps")
                        for o in range(2):
                            nc.tensor.matmul(
                                ps[:, o * 512:(o + 1) * 512],
                                lhsT=kt_sb[64 * o:64 * o + 64, j, ksl],
                                rhs=qt_sb[64 * o:64 * o + 64, j, qh],
                                start=True, stop=True,
                                tile_position=(64 * o, 0),
                            )
                        pt = ptpool.tile([128, 1024], BF16, tag="pt",
                                         name="pt")
                        nc.scalar.activation(pt[:, 0:ACT_COLS],
                                             ps[:, 0:ACT_COLS], Exp,
                                             scale=SCALE)
                        # Schraudolph: exp via affine into bf16 exponent
                        # bits (fp32->int16 convert + bitcast)
                        nc.vector.tensor_scalar(
                            pt[:, ACT_COLS:].bitcast(I16),
                            ps[:, ACT_COLS:], EXP_A, EXP_B, mult, add)
                        pts[kt] = pt

                    def ctx_stage(kt):
                        pt = pts.pop(kt)
                        for o in range(2):
                            nc.tensor.matmul(
                                pcs[o][:],
                                lhsT=vp_sb[:, kt, 2 * j + o, :],
                                rhs=pt[:, o * 512:(o + 1) * 512],
                                start=(kt == 0), stop=(kt == 15),
                            )

                    # previous block's reciprocal seeds at the top (GPSIMD
                    # polish runs during kt0-9) and its broadcast+normalize
                    # lands at kt10 (deps long since ready), so nothing
                    # downstream ever stalls the in-order PE queue
                    rinv_prev = newton_recip(pend[3]) if pend else None
                    for kt in range(16):
                        scores_stage(kt)
                        if kt == 10 and pend is not None:
                            finish_norm(pend[0], pend[1], pend[2],
                                        pend[3], rinv_prev)
                        if kt >= PIPE:
                            ctx_stage(kt - PIPE)
                    for kt in range(16 - PIPE, 16):
                        ctx_stage(kt)
                    cus = []
                    for o in range(2):
                        # sole reader of pc is this ACT copy -> pc-slot
                        # reuse WAR lands on the ACT sem (fast turnaround)
                        cu = cupool.tile([HD + 1, 512], F32, tag="cu",
                                         name="cu")
                        nc.scalar.copy(cu[:], pcs[o][:])
                        cus.append(cu)
                    return cus

                def finish_norm(n2, qc, j, cus, rinv):
                    # broadcast 1/denom (computed by newton_recip earlier in
                    # this block's stream) and normalize the context
                    q0 = n2 * 1024 + qc * 512
                    qsl = slice(q0, q0 + 512)
                    pb = pspool.tile([128, 1024], F32, tag="ps", name="pb")
                    for o in range(2):
                        # separate banks (cols 0-511 / 512-1023) so the two
                        # col-tiled broadcasts are independent groups
                        nc.tensor.matmul(
                            pb[64 * o:64 * o + 64, 512 * o:512 * o + 512],
                            lhsT=ones_sb[:], rhs=rinv[0:1, o, :],
                            start=True, stop=True,
                            tile_position=(0, 64 * o))
                    for o in range(2):
                        nc.vector.tensor_tensor(
                            out=ctx_sb[64 * o:64 * o + 64, j, qsl],
                            in0=cus[o][0:HD, :],
                            in1=pb[64 * o:64 * o + 64, 512 * o:512 * o + 512],
                            op=mult)

                def wo_qt(n2, qc, qt):
                    qg = n2 * 8 + qc * 4 + qt
                    ob = obpool.tile([128, D], BF16)
                    # both nn halves in one 2-bank psum tile; interleave
                    # the kc accumulation so consecutive matmuls alternate
                    # banks; the two PSUM->SBUF casts split across engines
                    po = pspool.tile([128, 1024], F32, tag="ps", name="po")
                    for kc in range(2):
                        for nn in range(2):
                            nc.tensor.matmul(
                                po[:, nn * 512:(nn + 1) * 512],
                                lhsT=ctx_sb[:, kc, qg * 128:(qg + 1) * 128],
                                rhs=wo_sb[:, kc, nn * 512:(nn + 1) * 512],
                                start=(kc == 0), stop=(kc == 1),
                            )
                    nc.scalar.copy(ob[:, 0:512], po[:, 0:512])
                    nc.vector.tensor_copy(ob[:, 512:1024], po[:, 512:1024])
                    nc.sync.dma_start(
                        out=out_e[qg * 128:(qg + 1) * 128, :], in_=ob[:])

                def wo_stage(n2, qc):
                    for qt in range(4):
                        wo_qt(n2, qc, qt)

                blocks = [(n2, qc, j) for n2 in range(2)
                          for qc in range(2) for j in range(2)]
                pend = None
                wo_pend = None
                for n2, qc, j in blocks:
                    cus = block_loop(n2, qc, j, pend)
                    # output projection for the (n2,qc) whose second pair
                    # was normalized mid-way through this block
                    if wo_pend is not None:
                        wo_stage(*wo_pend)
                        wo_pend = None
                    if pend is not None and pend[2] == 1:
                        wo_pend = (pend[0], pend[1])
                    pend = (n2, qc, j, cus)
                pn2, pqc, pj, pcus = pend
                if wo_pend is not None:
                    wo_stage(*wo_pend)
                finish_norm(pn2, pqc, pj, pcus,
                            newton_recip(pcus, eng=nc.vector))
                wo_stage(pn2, pqc)
    nc.compile()
    return nc



def _shard_inputs(x, Wq, Wk, Wv, Wo):
    """Build the 8 per-core input maps (host-side layout prep, bf16)."""
    import ml_dtypes

    bf16 = ml_dtypes.bfloat16
    in_maps = []
    xtb = [
        np.ascontiguousarray(
            x[b].T.reshape(8, 128, S).transpose(1, 0, 2)).astype(bf16)
        for b in range(B)
    ]
    for core in range(NCORES):
        b, g = divmod(core, GROUPS)
        gsl = slice(g * DL, (g + 1) * DL)
        wq = np.ascontiguousarray(
            Wq[:, gsl].reshape(8, 128, DL).transpose(1, 0, 2)).astype(bf16)
        wk = np.ascontiguousarray(
            Wk[:, gsl].reshape(8, 128, DL).transpose(1, 0, 2)).astype(bf16)
        wv = np.ascontiguousarray(
            Wv[:, gsl].reshape(8, 128, DL).transpose(1, 0, 2)).astype(bf16)
        wo = np.ascontiguousarray(
            Wo[gsl, :].reshape(2, 128, D).transpose(1, 0, 2)).astype(bf16)
        in_maps.append(
            {"xt": xtb[b], "wq": wq, "wk": wk, "wv": wv, "wo": wo})
    return in_maps


def _gather(results, bo):
    out = np.zeros((B, S, D), dtype=np.float32)
    for core in range(NCORES):
        b = core // GROUPS
        out[b] += results[core]["out"].astype(np.float32)
    out += bo.astype(np.float32)
    return out


def _run_device(x, Wq, Wk, Wv, Wo, bo, trace=False, tmpdir=None):
    from concourse.bass_utils import run_bass_kernel_spmd

    nc = _build_graph()
    in_maps = _shard_inputs(x, Wq, Wk, Wv, Wo)
    bkr = run_bass_kernel_spmd(
        nc, in_maps, core_ids=list(range(NCORES)), trace=trace, tmpdir=tmpdir)
    return _gather(bkr.results, bo), bkr


def _reference_numpy(x, mask, Wq, bq, Wk, bk, Wv, bv, Wo, bo):
    """Exact fallback for inputs outside the hardcoded spec."""
    b, s, d = x.shape
    h = H if d % H == 0 else 1
    hd = d // h
    q = (x @ Wq + bq).reshape(b, s, h, hd).transpose(0, 2, 1, 3)
    k = (x @ Wk + bk).reshape(b, s, h, hd).transpose(0, 2, 1, 3)
    v = (x @ Wv + bv).reshape(b, s, h, hd).transpose(0, 2, 1, 3)
    scores = np.einsum("bhqd,bhkd->bhqk", q, k) * (hd ** -0.5)
    scores = np.where(mask[:, None, None, :] == 0, -np.inf, scores)
    scores -= scores.max(axis=-1, keepdims=True)
    e = np.exp(scores)
    attn = e / e.sum(axis=-1, keepdims=True)
    ctx = np.einsum("bhqk,bhkd->bhqd", attn, v)
    ctx = ctx.transpose(0, 2, 1, 3).reshape(b, s, d)
    return (ctx @ Wo + bo).astype(np.float32)


def kernel(x, mask, Wq, bq, Wk, bk, Wv, bv, Wo, bo):
    x = np.asarray(x, dtype=np.float32)
    mask = np.asarray(mask)
    Wq, bq = np.asarray(Wq, np.float32), np.asarray(bq, np.float32)
    Wk, bk = np.asarray(Wk, np.float32), np.asarray(bk, np.float32)
    Wv, bv = np.asarray(Wv, np.float32), np.asarray(bv, np.float32)
    Wo, bo = np.asarray(Wo, np.float32), np.asarray(bo, np.float32)

    general = (
        x.shape != (B, S, D)
        or not np.all(mask == 1)
        or any(np.any(t != 0) for t in (bq, bk, bv))
    )
    if general:
        return _reference_numpy(x, mask, Wq, bq, Wk, bk, Wv, bv, Wo, bo)

    out, _ = _run_device(x, Wq, Wk, Wv, Wo, bo)
    return out


# revision 45
# speedup vs baseline: 1.1561x; 1.1561x over previous
"""Trainium2 8-core kernel for nn_AttentionMechanism_51049981281163.

Reference module: multi-head attention, B=2, S=2048, D=1024, H=16 heads,
head_dim=64, fp32, mask all-ones, biases all-zero.

Sharding: batch x head-group tensor parallel. Core c handles batch b=c//4
and head group g=c%4 (4 heads = 256 of the 1024 hidden dims). Wq/Wk/Wv are
split column-wise, Wo row-wise; each core computes a partial [S, D] output
and the host sums the 4 partials per batch (the "unshard" of row-parallel
Wo) and adds bo.

Device kernel (per core), bf16 matmul inputs with fp32 PSUM accumulate:
  - host passes x[b].T pre-tiled, so projections need no on-device transpose
  - Phase A streams the V projection against the chunked xt DMA (PE does
    real work during the input transfer; no dummy warm-up matmuls), then
    runs Q/K through the same rotating 8-bank PSUM pool; PSUM->SBUF casts
    are split between ScalarE and VectorE
  - QT/KT in [head_dim, S] layout; V in [S, head_dim] layout with a ones
    column per head (softmax denominators ride along as context row 64)
  - scores computed transposed [k, q]; the two heads of a pair use T0/T8
    64-row PE tiles into one two-bank PSUM tile
  - softmax exp is SPLIT between engines: half the k-tiles use ScalarE
    exp, half use a VectorE Schraudolph bit-trick (affine into the bf16
    exponent field via fp32->int16 convert, bitcast to bf16); the mean
    relative error of the trick is calibrated to ~0 so softmax
    normalization cancels most of it
  - 1/denominator = exp(-ln(denom)) on ScalarE (Ln and Exp share one ACT
    table set -> no table reloads), then ONE K=2 block-diagonal matmul
    broadcasts both heads' reciprocal rows across partitions; the final
    normalization multiply reads that PSUM tile directly
  - output projection from the context (already in lhsT layout) x Wo shard
"""

import sys

sys.path.insert(0, "/opt/trn_rl_repo")

import numpy as np

B, S, D, H = 2, 2048, 1024, 16
HD = D // H          # 64
SCALE = HD ** -0.5
NCORES = 8
GROUPS = 4           # head groups (cores per batch)
HPG = H // GROUPS    # 4 heads per group/core
DL = HPG * HD        # 256 local hidden dims per core
VW = HD + 1          # V block width incl. ones column

# Schraudolph exp in bf16 bit domain: i16 = x*SCALE*128*log2(e) + bias;
# bitcast(i16) ~= exp(x*SCALE).  Bias centered between round/trunc convert
# semantics; mean rel err ~0 (cancels in softmax), rms ~1.8%.
_LOG2E = 1.4426950408889634
EXP_A = SCALE * 128.0 * _LOG2E          # 23.0831...
EXP_B = 127.0 * 128.0 - 7.15            # 16248.85
# every score tile's exp is split by columns: ScalarE does [0, ACT_COLS)
# exactly, VectorE does the rest via the bit-trick — both engines work on
# the SAME k-tile concurrently at ~0.6us/tile, so exp always completes
# before the in-order PE queue reaches the context matmuls
ACT_COLS = 576
# Newton reciprocal: seed 1/d ~= bitcast(K - bits(d)), then r*(2 - d*r)
RCP_K = float(0x7EF0A3D7)
# context matmuls trail the scores stream by PIPE k-tiles so the exp
# latency never stalls the in-order PE queue (PIPE=3 regresses: the
# 3-slot scores PSUM pool then throttles the scores stream instead)
PIPE = 2


def _build_graph():
    import concourse.tile as tile
    from concourse import bacc, mybir

    F32 = mybir.dt.float32
    BF16 = mybir.dt.bfloat16
    I16 = mybir.dt.int16
    Exp = mybir.ActivationFunctionType.Exp
    Ln = mybir.ActivationFunctionType.Ln
    mult = mybir.AluOpType.mult
    add = mybir.AluOpType.add

    nc = bacc.Bacc("TRN2")

    # x[b].T tiled: [p, c, s] = x[b][s, c*128+p]
    xt_e = nc.declare_dram_parameter("xt", [128, 8, S], BF16, isOutput=False)
    # W[:, gsl] tiled: [p, c, d] = W[c*128+p, g*256+d]
    wq_e = nc.declare_dram_parameter("wq", [128, 8, DL], BF16, isOutput=False)
    wk_e = nc.declare_dram_parameter("wk", [128, 8, DL], BF16, isOutput=False)
    wv_e = nc.declare_dram_parameter("wv", [128, 8, DL], BF16, isOutput=False)
    # Wo[gsl, :] tiled: [p, kc, dd] = Wo[g*256+kc*128+p, dd]
    wo_e = nc.declare_dram_parameter("wo", [128, 2, D], BF16, isOutput=False)
    out_e = nc.declare_dram_parameter("out", [S, D], BF16, isOutput=True)

    with nc.allow_low_precision(reason="bf16 compute, 2e-2 tolerance"), \
         tile.TileContext(nc) as tc:
        with tc.tile_pool(name="big", bufs=1) as big:
            xt_sb = big.tile([128, 8, S], BF16)
            wq_sb = big.tile([128, 8, DL], BF16)
            wk_sb = big.tile([128, 8, DL], BF16)
            wv_sb = big.tile([128, 8, DL], BF16)
            wo_sb = big.tile([128, 2, D], BF16)
            qt_sb = big.tile([128, 2, S], BF16)
            kt_sb = big.tile([128, 2, S], BF16)
            vp_sb = big.tile([128, 16, HPG, VW], BF16)
            ctx_sb = big.tile([128, 2, S], BF16)
            ones_sb = big.tile([1, 64], BF16)

            nc.vector.memset(ones_sb[:], 1.0)
            nc.vector.memset(vp_sb[:, :, :, HD], 1.0)

            # DMA order = consumption order: wv + x chunks feed the
            # streamed V projection, then Q/K/Wo weights. Inputs split
            # across the two HWDGE queues (sync + scalar) for bandwidth;
            # ScalarE is idle this early so the issue cost is free.
            nc.sync.dma_start(out=wv_sb[:], in_=wv_e[:])
            for c in range(8):
                eng = nc.sync if c % 2 == 0 else nc.scalar
                if c == 0:
                    # first chunk in 4 pieces so the first V matmuls can
                    # start as soon as ~a quarter of it lands
                    for p in range(4):
                        psl = slice(p * 512, (p + 1) * 512)
                        eng.dma_start(out=xt_sb[:, 0, psl],
                                      in_=xt_e[:, 0, psl])
                else:
                    eng.dma_start(out=xt_sb[:, c, :], in_=xt_e[:, c, :])
            nc.scalar.dma_start(out=wk_sb[:], in_=wk_e[:])
            nc.sync.dma_start(out=wq_sb[:], in_=wq_e[:])
            nc.scalar.dma_start(out=wo_sb[:], in_=wo_e[:])

            # ---- Phase A: projections through one rotating PSUM pool ----
            # V streams against the xt DMA: each arriving 128-row chunk of
            # x.T enables 16 matmuls, keeping PE busy during the transfer.
            with tc.tile_pool(name="pa", bufs=8, space="PSUM") as pa:
                pv = [pa.tile([128, 512], F32, tag="pa", name="pv")
                      for _ in range(8)]
                # one accumulation group per PSUM bank (start zeroes the
                # whole bank): the kt pair sharing a tile is one group
                for c in range(8):
                    for kt in range(16):
                        nc.tensor.matmul(
                            pv[kt // 2][:, (kt % 2) * 256:(kt % 2) * 256 + 256],
                            lhsT=xt_sb[:, c, kt * 128:(kt + 1) * 128],
                            rhs=wv_sb[:, c, :],
                            start=(c == 0 and kt % 2 == 0),
                            stop=(c == 7 and kt % 2 == 1),
                            skip_group_check=True,
                        )
                for kt in range(16):
                    src = pv[kt // 2][:, (kt % 2) * 256:(kt % 2) * 256 + 256]
                    dst = vp_sb[:, kt, :, 0:HD]
                    srcr = src.rearrange("p (h d) -> p h d", h=HPG)
                    if kt % 2 == 0:
                        nc.vector.tensor_copy(dst, srcr)
                    else:
                        nc.scalar.copy(dst, srcr)
                # K then Q through the same pool; allocations recycle banks
                # as the V (then K) casts drain. Two accumulation chains
                # are interleaved so consecutive matmuls hit different
                # PSUM banks (same-bank accumulation serializes at the
                # full fill+drain latency instead of the N-cycle rate).
                for w_sb, o_sb in ((wk_sb, kt_sb), (wq_sb, qt_sb)):
                    for m in range(2):
                        for h2 in range(2):
                            pp = [pa.tile([128, 512], F32, tag="pa",
                                          name="pp") for _ in range(2)]
                            for c in range(8):
                                for i in range(2):
                                    n4 = 2 * h2 + i
                                    nc.tensor.matmul(
                                        pp[i][:],
                                        lhsT=w_sb[:, c, m * 128:(m + 1) * 128],
                                        rhs=xt_sb[:, c,
                                                  n4 * 512:(n4 + 1) * 512],
                                        start=(c == 0), stop=(c == 7),
                                    )
                            for i in range(2):
                                n4 = 2 * h2 + i
                                dst = o_sb[:, m, n4 * 512:(n4 + 1) * 512]
                                if i == 0:
                                    nc.vector.tensor_copy(dst, pp[i][:])
                                else:
                                    nc.scalar.copy(dst, pp[i][:])

            # ---- Phase B: attention + output projection ----
            with tc.tile_pool(name="ps", bufs=3, space="PSUM") as pspool, \
                 tc.tile_pool(name="pc", bufs=2, space="PSUM") as pcpool, \
                 tc.tile_pool(name="pt", bufs=5) as ptpool, \
                 tc.tile_pool(name="cu", bufs=6) as cupool, \
                 tc.tile_pool(name="sm", bufs=4) as smpool, \
                 tc.tile_pool(name="ob", bufs=3) as obpool:
                def newton_recip(cus, eng=None):
                    # 1/denom for both heads: bit-hack seed r0 =
                    # bitcast(K - bits(d)) via fp32 affine on the int32 view
                    # + convert-on-write (VectorE), then one Newton step
                    # r0*(2 - d*r0) on the otherwise-idle GPSIMD engine ->
                    # ~0.2% worst case, squared from ~4%. Emitted inside the
                    # NEXT block's kt0-3 window, where the DVE has no exp
                    # tiles queued.
                    r0i = smpool.tile([1, 2, 512], mybir.dt.int32, tag="ri")
                    dc = smpool.tile([1, 2, 512], F32, tag="dc")
                    nu = smpool.tile([1, 2, 512], F32, tag="nu")
                    nv = smpool.tile([1, 2, 512], F32, tag="nv")
                    rinv = smpool.tile([1, 2, 512], BF16, tag="rinv")
                    for o in range(2):
                        # single-src DVE ops may cross partition bases;
                        # stage the denominator row at partition 0
                        nc.vector.tensor_copy(dc[0:1, o, :],
                                              cus[o][HD:HD + 1, :])
                    nc.vector.tensor_scalar(
                        r0i[:], dc[:].bitcast(mybir.dt.int32),
                        -1.0, RCP_K, mult, add)
                    r0 = r0i[:].bitcast(F32)
                    if eng is None:
                        eng = nc.gpsimd
                    if eng is nc.gpsimd:
                        eng.tensor_mul(nu[:], dc[:], r0)
                    else:
                        eng.tensor_tensor(out=nu[:], in0=dc[:], in1=r0,
                                          op=mult)
                    eng.tensor_scalar(nv[:], nu[:], -1.0, 2.0, mult, add)
                    if eng is nc.gpsimd:
                        eng.tensor_mul(rinv[:], r0, nv[:])
                    else:
                        eng.tensor_tensor(out=rinv[:], in0=r0, in1=nv[:],
                                          op=mult)
                    return rinv

                def block_loop(n2, qc, j, pend):
                    # heads 2j/2j+1, q-chunk of 512. Both heads' scores land
                    # in ONE [128,1024] psum tile (separate banks). The
                    # context matmuls for k-tile kt are emitted after the
                    # scores for kt+PIPE, so the exp latency (ScalarE or
                    # VectorE) is hidden behind later scores instead of
                    # stalling the in-order PE queue.
                    q0 = n2 * 1024 + qc * 512
                    qh = slice(q0, q0 + 512)
                    pcs = []
                    for _ in range(2):
                        pcs.append(pcpool.tile([HD + 1, 512], F32, tag="pc",
                                               name="pc"))
                    pts = {}

                    def scores_stage(kt):
                        ksl = slice(kt * 128, (kt + 1) * 128)
                        ps = pspool.tile([128, 1024], F32, tag="ps", name="ps")
                        for o in range(2):
                            nc.tensor.matmul(
                                ps[:, o * 512:(o + 1) * 512],
                                lhsT=kt_sb[64 * o:64 * o + 64, j, ksl],
                                rhs=qt_sb[64 * o:64 * o + 64, j, qh],
                                start=True, stop=True,
                                tile_position=(64 * o, 0),
                            )
                        pt = ptpool.tile([128, 1024], BF16, tag="pt",
                                         name="pt")
                        nc.scalar.activation(pt[:, 0:ACT_COLS],
                                             ps[:, 0:ACT_COLS], Exp,
                                             scale=SCALE)
                        # Schraudolph: exp via affine into bf16 exponent
                        # bits (fp32->int16 convert + bitcast)
                        nc.vector.tensor_scalar(
                            pt[:, ACT_COLS:].bitcast(I16),
                            ps[:, ACT_COLS:], EXP_A, EXP_B, mult, add)
                        pts[kt] = pt

                    def ctx_stage(kt):
                        pt = pts.pop(kt)
                        for o in range(2):
                            nc.tensor.matmul(
                                pcs[o][:],
                                lhsT=vp_sb[:, kt, 2 * j + o, :],
                                rhs=pt[:, o * 512:(o + 1) * 512],
                                start=(kt == 0), stop=(kt == 15),
                            )

                    # previous block's reciprocal seeds at the top (GPSIMD
                    # polish runs during kt0-9) and its broadcast+normalize
                    # lands at kt10 (deps long since ready), so nothing
                    # downstream ever stalls the in-order PE queue
                    rinv_prev = newton_recip(pend[3]) if pend else None
                    for kt in range(16):
                        scores_stage(kt)
                        if kt == 10 and pend is not None:
                            finish_norm(pend[0], pend[1], pend[2],
                                        pend[3], rinv_prev)
                        if kt >= PIPE:
                            ctx_stage(kt - PIPE)
                    for kt in range(16 - PIPE, 16):
                        ctx_stage(kt)
                    cus = []
                    for o in range(2):
                        # sole reader of pc is this ACT copy -> pc-slot
                        # reuse WAR lands on the ACT sem (fast turnaround)
                        cu = cupool.tile([HD + 1, 512], F32, tag="cu",
                                         name="cu")
                        nc.scalar.copy(cu[:], pcs[o][:])
                        cus.append(cu)
                    return cus

                def finish_norm(n2, qc, j, cus, rinv):
                    # broadcast 1/denom (computed by newton_recip earlier in
                    # this block's stream) and normalize the context
                    q0 = n2 * 1024 + qc * 512
                    qsl = slice(q0, q0 + 512)
                    pb = pspool.tile([128, 1024], F32, tag="ps", name="pb")
                    for o in range(2):
                        # separate banks (cols 0-511 / 512-1023) so the two
                        # col-tiled broadcasts are independent groups
                        nc.tensor.matmul(
                            pb[64 * o:64 * o + 64, 512 * o:512 * o + 512],
                            lhsT=ones_sb[:], rhs=rinv[0:1, o, :],
                            start=True, stop=True,
                            tile_position=(0, 64 * o))
                    for o in range(2):
                        nc.vector.tensor_tensor(
                            out=ctx_sb[64 * o:64 * o + 64, j, qsl],
                            in0=cus[o][0:HD, :],
                            in1=pb[64 * o:64 * o + 64, 512 * o:512 * o + 512],
                            op=mult)

                def wo_qt(n2, qc, qt):
                    qg = n2 * 8 + qc * 4 + qt
                    ob = obpool.tile([128, D], BF16)
                    # both nn halves in one 2-bank psum tile; interleave
                    # the kc accumulation so consecutive matmuls alternate
                    # banks; the two PSUM->SBUF casts split across engines
                    po = pspool.tile([128, 1024], F32, tag="ps", name="po")
                    for kc in range(2):
                        for nn in range(2):
                            nc.tensor.matmul(
                                po[:, nn * 512:(nn + 1) * 512],
                                lhsT=ctx_sb[:, kc, qg * 128:(qg + 1) * 128],
                                rhs=wo_sb[:, kc, nn * 512:(nn + 1) * 512],
                                start=(kc == 0), stop=(kc == 1),
                            )
                    nc.scalar.copy(ob[:, 0:512], po[:, 0:512])
                    nc.vector.tensor_copy(ob[:, 512:1024], po[:, 512:1024])
                    nc.sync.dma_start(
                        out=out_e[qg * 128:(qg + 1) * 128, :], in_=ob[:])

                def wo_stage(n2, qc):
                    for qt in range(4):
                        wo_qt(n2, qc, qt)

                blocks = [(n2, qc, j) for n2 in range(2)
                          for qc in range(2) for j in range(2)]
                pend = None
                wo_pend = None
                for n2, qc, j in blocks:
                    cus = block_loop(n2, qc, j, pend)
                    # output projection for the (n2,qc) whose second pair
                    # was normalized mid-way through this block
                    if wo_pend is not None:
                        wo_stage(*wo_pend)
                        wo_pend = None
                    if pend is not None and pend[2] == 1:
                        wo_pend = (pend[0], pend[1])
                    pend = (n2, qc, j, cus)
                pn2, pqc, pj, pcus = pend
                if wo_pend is not None:
                    wo_stage(*wo_pend)
                finish_norm(pn2, pqc, pj, pcus,
                            newton_recip(pcus, eng=nc.vector))
                wo_stage(pn2, pqc)
    nc.compile()
    return nc



def _shard_inputs(x, Wq, Wk, Wv, Wo):
    """Build the 8 per-core input maps (host-side layout prep, bf16)."""
    import ml_dtypes

    bf16 = ml_dtypes.bfloat16
    in_maps = []
    xtb = [
        np.ascontiguousarray(
            x[b].T.reshape(8, 128, S).transpose(1, 0, 2)).astype(bf16)
        for b in range(B)
    ]
    for core in range(NCORES):
        b, g = divmod(core, GROUPS)
        gsl = slice(g * DL, (g + 1) * DL)
        wq = np.ascontiguousarray(
            Wq[:, gsl].reshape(8, 128, DL).transpose(1, 0, 2)).astype(bf16)
        wk = np.ascontiguousarray(
            Wk[:, gsl].reshape(8, 128, DL).transpose(1, 0, 2)).astype(bf16)
        wv = np.ascontiguousarray(
            Wv[:, gsl].reshape(8, 128, DL).transpose(1, 0, 2)).astype(bf16)
        wo = np.ascontiguousarray(
            Wo[gsl, :].reshape(2, 128, D).transpose(1, 0, 2)).astype(bf16)
        in_maps.append(
            {"xt": xtb[b], "wq": wq, "wk": wk, "wv": wv, "wo": wo})
    return in_maps


def _gather(results, bo):
    out = np.zeros((B, S, D), dtype=np.float32)
    for core in range(NCORES):
        b = core // GROUPS
        out[b] += results[core]["out"].astype(np.float32)
    out += bo.astype(np.float32)
    return out


def _run_device(x, Wq, Wk, Wv, Wo, bo, trace=False, tmpdir=None):
    from concourse.bass_utils import run_bass_kernel_spmd

    nc = _build_graph()
    in_maps = _shard_inputs(x, Wq, Wk, Wv, Wo)
    bkr = run_bass_kernel_spmd(
        nc, in_maps, core_ids=list(range(NCORES)), trace=trace, tmpdir=tmpdir)
    return _gather(bkr.results, bo), bkr


def _reference_numpy(x, mask, Wq, bq, Wk, bk, Wv, bv, Wo, bo):
    """Exact fallback for inputs outside the hardcoded spec."""
    b, s, d = x.shape
    h = H if d % H == 0 else 1
    hd = d // h
    q = (x @ Wq + bq).reshape(b, s, h, hd).transpose(0, 2, 1, 3)
    k = (x @ Wk + bk).reshape(b, s, h, hd).transpose(0, 2, 1, 3)
    v = (x @ Wv + bv).reshape(b, s, h, hd).transpose(0, 2, 1, 3)
    scores = np.einsum("bhqd,bhkd->bhqk", q, k) * (hd ** -0.5)
    scores = np.where(mask[:, None, None, :] == 0, -np.inf, scores)
    scores -= scores.max(axis=-1, keepdims=True)
    e = np.exp(scores)
    attn = e / e.sum(axis=-1, keepdims=True)
    ctx = np.einsum("bhqk,bhkd->bhqd", attn, v)
    ctx = ctx.transpose(0, 2, 1, 3).reshape(b, s, d)
    return (ctx @ Wo + bo).astype(np.float32)


def kernel(x, mask, Wq, bq, Wk, bk, Wv, bv, Wo, bo):
    x = np.asarray(x, dtype=np.float32)
    mask = np.asarray(mask)
    Wq, bq = np.asarray(Wq, np.float32), np.asarray(bq, np.float32)
    Wk, bk = np.asarray(Wk, np.float32), np.asarray(bk, np.float32)
    Wv, bv = np.asarray(Wv, np.float32), np.asarray(bv, np.float32)
    Wo, bo = np.asarray(Wo, np.float32), np.asarray(bo, np.float32)

    general = (
        x.shape != (B, S, D)
        or not np.all(mask == 1)
        or any(np.any(t != 0) for t in (bq, bk, bv))
    )
    if general:
        return _reference_numpy(x, mask, Wq, bq, Wk, bk, Wv, bv, Wo, bo)

    out, _ = _run_device(x, Wq, Wk, Wv, Wo, bo)
    return out
